# revision 52
# baseline (speedup 1.0000x reference)
"""Trainium2 Bass kernel for multi-head self-attention (B=2, N=4096, C=512, H=8).

Sharding: 8 cores = 2 batches x 4 head-pairs. Core c handles batch c//4 and
heads {2*(c%4), 2*(c%4)+1}. Each core computes its two heads' attention over
all 4096 tokens and a partial output projection restricted to its heads' 128
channels; the host sums the 4 partials per batch (the tensor-parallel proj
all-reduce) and adds b_proj.

Host ships x pre-transposed as fp16 xT [512, 4096] and fp16 weights, so the
device prologue is just DMA + qkv matmuls (no PE x-transposes, no DVE casts).
All matmul operands are fp16 (1 cycle/row on the PE vs 2 for f32r).

The softmax exp stream — the steady-state bottleneck — is split across BOTH
transcendental-capable paths by head: head0's exp runs on ScalarE (LUT exp),
head1's on VectorE via a Schraudolph-style fast exp (one fused mult+add
tensor_scalar emitting fp16 bit patterns through a saturating uint16 cast).
Each head's chain PE(scores) -> exp engine -> PE(PV) only ever waits on its
own exp engine. The kt loop is software-pipelined (PV of kt-1 issues after
scores of kt) so the in-order PE queue never head-of-line blocks on an exp.

All P values carry a free 2^-5 scale (folded into the exp bias / fast-exp
constant) so softmax denominators fit fp16; the scale cancels in P/sum and
lets the sums->partition transpose matmuls run in fp16 (K=1 fp32 matmuls
double-pump the PE).

Device dataflow per core (scores never touch DRAM):
  kT = wk^T @ xT           [128, 4096]   (rows 0-63 head0, 64-127 head1)
  qT likewise              [128, 4096]
  v  = transpose(wv^T @ xT) per 128-token tile: [Vh0 | 1 | Vh1 | 1] [128,130]
  per 512-query chunk, per 128-key tile:
    S^T = kT_tile^T @ qT   (two concurrent K=64 row-packed matmuls)
    P^T = 2^-5 exp(SCALE * S^T)  (ScalarE for h0, DVE fast-exp for h1)
    PV += [V|1]^T @ P^T    (PSUM accumulate; row 64 = softmax denominators)
  proj: out_qtile = (outT_h0^T @ wp_h0) * (1/sum_h0) + (outT_h1^T @ wp_h1) * (1/sum_h1)
"""

import math
import os
import sys

if "/opt/trn_rl_repo" not in sys.path:
    sys.path.insert(0, "/opt/trn_rl_repo")

import numpy as np

import concourse.bass as bass
import concourse.mybir as mybir
import concourse.tile as tile
from concourse import bacc
from concourse.masks import make_identity

B, N, C, H = 2, 4096, 512, 8
D = C // H
SCALE = D**-0.5
F32 = mybir.dt.float32
F16 = mybir.dt.float16
# uint16 so the DVE float->unsigned cast saturates negatives to +0.0
# (an out-of-range-low fast-exp result must clamp to P=0, and negative
# int16 bit patterns would decode as fp16 NaN/-huge)
U16 = mybir.dt.uint16

# All P values are scaled by 2^-SHIFT (free via the exp bias) so softmax
# denominators stay well inside fp16 range; the scale cancels in P/sum.
SHIFT = 5
SUMS16 = True

# Schraudolph fast-exp constants for fp16 bit patterns:
#   bits = round(1024 * (log2(e)*SCALE*s + 15 - SHIFT - c))
SCH_A = 1024.0 * SCALE * math.log2(math.e)
SCH_C = float(os.environ.get("ATTN_SCH_C", "0.0579"))
SCH_B = 1024.0 * (15.0 - SHIFT - SCH_C)
EXP_BIAS = -SHIFT * math.log(2.0)

MM_DT_NAME = "f16, exp split ScalarE(h0)/DVE-fast-exp(h1)"


def build(tokens=N, timing=False):
    T = tokens
    n_xt = T // 128  # 128-token tiles (key tiles / v tiles)
    n_g = T // 1024  # 1024-token groups for kT/qT/xT
    n_qc = T // 512  # query chunks

    EXP = mybir.ActivationFunctionType.Exp
    MUL = mybir.AluOpType.mult
    ADD = mybir.AluOpType.add

    nc = bacc.Bacc(None)
    # x arrives host-pre-transposed fp16: xt[c, t] = x[t, c]
    xt = nc.dram_tensor("xt", [C, T], F16, kind="ExternalInput")
    out = nc.dram_tensor("out", [T, C], F32, kind="ExternalOutput")
    # weights arrive host-pre-transposed fp16 into SBUF layout [128, 512]:
    # w*_[p, kc*128 + j] = w[kc*128 + p, j]
    wq = nc.dram_tensor("wq", [128, 512], F16, kind="ExternalInput")
    wk = nc.dram_tensor("wk", [128, 512], F16, kind="ExternalInput")
    wv = nc.dram_tensor("wv", [128, 512], F16, kind="ExternalInput")
    wp = nc.dram_tensor("wp", [128, C], F16, kind="ExternalInput")

    with tile.TileContext(nc) as tc:
        with tc.tile_pool(name="persist", bufs=1) as pp:
            ident = pp.tile([128, 128], F32, tag="ident")
            make_identity(nc, ident[:])
            ident16 = pp.tile([128, 128], F16, tag="ident16")
            nc.vector.tensor_copy(ident16[:], ident[:])
            ebias = pp.tile([128, 1], F32, tag="ebias")
            nc.gpsimd.memset(ebias[:], EXP_BIAS)

            w_sbs = {}
            for wname, wdram in (("wq", wq), ("wk", wk), ("wv", wv), ("wp", wp)):
                w_sb = pp.tile([128, 512], F16, tag=f"{wname}_sb", name=f"{wname}_sb")
                nc.sync.dma_start(out=w_sb[:], in_=wdram[:, :])
                w_sbs[wname] = w_sb
            wq_sb, wk_sb, wv_sb, wp_sb = (
                w_sbs["wq"],
                w_sbs["wk"],
                w_sbs["wv"],
                w_sbs["wp"],
            )

            kT = [
                pp.tile([128, 1024], F16, tag=f"kT{g}", name=f"kT{g}")
                for g in range(n_g)
            ]
            qT = [
                pp.tile([128, 1024], F16, tag=f"qT{g}", name=f"qT{g}")
                for g in range(n_g)
            ]
            v = [
                pp.tile([128, 130], F16, tag=f"v{t}", name=f"v{t}")
                for t in range(n_xt)
            ]
            for t in range(n_xt):
                nc.gpsimd.memset(v[t][:, 64:65], 1.0)
                nc.gpsimd.memset(v[t][:, 129:130], 1.0)
            outT = [
                pp.tile([128, 512], F16, tag=f"outT{s}", name=f"outT{s}")
                for s in range(n_qc)
            ]

            def qt_slice(qc):
                return qT[qc // 2][:, (qc % 2) * 512 : (qc % 2) * 512 + 512]

            def scores_step(qc, kt, psS, ptp):
                # head0's chain runs PE->ScalarE->PE, head1's PE->DVE->PE:
                # each PV matmul depends on exactly one exp engine, so a
                # hiccup on one engine never stalls the other head's chain.
                sc0 = psS.tile([128, 512], F32, tag="sc0", name="sc0")
                sc1 = psS.tile([128, 512], F32, tag="sc1", name="sc1")
                kslc = kT[kt // 8][:, (kt % 8) * 128 : (kt % 8 + 1) * 128]
                qslc = qt_slice(qc)
                nc.tensor.matmul(
                    sc0[:],
                    kslc[0:64, :],
                    qslc[0:64, :],
                    start=True,
                    stop=True,
                    tile_position=(0, 0),
                )
                nc.tensor.matmul(
                    sc1[:],
                    kslc[64:128, :],
                    qslc[64:128, :],
                    start=True,
                    stop=True,
                    tile_position=(64, 0),
                )
                pt0 = ptp.tile([128, 512], U16, tag="pt0", name="pt0")
                pt1 = ptp.tile([128, 512], U16, tag="pt1", name="pt1")
                nc.scalar.activation(
                    pt0[:].bitcast(F16), sc0[:], EXP, bias=ebias[:], scale=SCALE
                )
                # fast exp: fp16 bits = round(SCH_A * s + SCH_B)
                nc.vector.tensor_scalar(pt1[:], sc1[:], SCH_A, SCH_B, MUL, ADD)
                return pt0, pt1

            def pv_step(kt, pts, pv0, pv1, n_last):
                pt0, pt1 = pts
                nc.tensor.matmul(
                    pv0[:],
                    v[kt][:, 0:65],
                    pt0[:].bitcast(F16),
                    start=(kt == 0),
                    stop=(kt == n_last),
                )
                nc.tensor.matmul(
                    pv1[:],
                    v[kt][:, 65:130],
                    pt1[:].bitcast(F16),
                    start=(kt == 0),
                    stop=(kt == n_last),
                )

            def drain(qc, pv0, pv1, smp):
                sums = smp.tile([1, 1024], F16, tag="sums", name="sums")
                # h0 drains on ScalarE, h1 on DVE (keeps both exp engines'
                # side-work symmetric)
                nc.scalar.copy(outT[qc][0:64, :], pv0[0:64, :])
                nc.scalar.copy(sums[0:1, 0:512], pv0[64:65, :])
                nc.vector.tensor_copy(outT[qc][64:128, :], pv1[0:64, :])
                nc.vector.tensor_copy(sums[0:1, 512:1024], pv1[64:65, :])
                return sums

            def proj_qtile(qc, qs, sums, smp, osp, psT):
                i = qc * 4 + qs
                ta = psT.tile([128, 512], F32, tag="ta", name="ta")
                tb = psT.tile([128, 512], F32, tag="tb", name="tb")
                # denominators -> partition layout via K=1 fp16 matmuls
                nc.tensor.matmul(
                    ta[:, 0:1],
                    sums[0:1, qs * 128 : (qs + 1) * 128],
                    ident16[0:1, 0:1],
                    start=True,
                    stop=True,
                )
                nc.tensor.matmul(
                    ta[:, 1:2],
                    sums[0:1, 512 + qs * 128 : 512 + (qs + 1) * 128],
                    ident16[0:1, 0:1],
                    start=True,
                    stop=True,
                )
                rc = smp.tile([128, 2], F32, tag="recip", name="rc")
                nc.vector.reciprocal(rc[:], ta[:, 0:2])
                oslc = outT[qc][:, qs * 128 : (qs + 1) * 128]
                nc.tensor.matmul(
                    ta[:],
                    oslc[0:64, :],
                    wp_sb[0:64, :],
                    start=True,
                    stop=True,
                    tile_position=(0, 0),
                )
                nc.tensor.matmul(
                    tb[:],
                    oslc[64:128, :],
                    wp_sb[64:128, :],
                    start=True,
                    stop=True,
                    tile_position=(64, 0),
                )
                t0 = osp.tile([128, 512], F32, tag="t0", name="t0")
                nc.scalar.activation(
                    t0[:], ta[:], mybir.ActivationFunctionType.Copy,
                    bias=0.0, scale=rc[:, 0:1],
                )
                ob = osp.tile([128, 512], F32, tag="ob", name="ob")
                nc.vector.scalar_tensor_tensor(
                    ob[:], tb[:], rc[:, 1:2], t0[:], op0=MUL, op1=ADD
                )
                nc.sync.dma_start(out=out[i * 128 : (i + 1) * 128, :], in_=ob[:])

            with tc.tile_pool(name="ptp", bufs=6) as ptp, tc.tile_pool(
                name="smp", bufs=2
            ) as smp, tc.tile_pool(name="osp", bufs=2) as osp, tc.tile_pool(
                name="psS", bufs=2, space="PSUM"
            ) as psS, tc.tile_pool(name="psV", bufs=1, space="PSUM") as psV:
                pv0_0 = psV.tile([65, 512], F32, tag="pv0", name="pv0")
                pv1_0 = psV.tile([65, 512], F32, tag="pv1", name="pv1")

                # ---- prologue: produce kT/qT/v per 1024-token group, with
                # qc=0's attention interleaved so the exp engines start early
                with tc.tile_pool(name="ldp", bufs=2) as ldp, tc.tile_pool(
                    name="psA", bufs=1, space="PSUM"
                ) as psA:
                    prev_pts = None
                    for g in range(n_g):
                        xg = ldp.tile([128, 4096], F16, tag="xload", name="xg")
                        nc.sync.dma_start(
                            out=xg[:].rearrange("p (c w) -> p c w", c=4),
                            in_=xt[:, g * 1024 : (g + 1) * 1024].rearrange(
                                "(c p) w -> p c w", c=4
                            ),
                        )
                        vts = ldp.tile([128, 1024], F16, tag="vts", name="vts")
                        for h in range(2):
                            hs = slice(h * 512, h * 512 + 512)
                            for w_sb, dst, copy_eng in (
                                (wk_sb, kT[g], nc.scalar.copy),
                                (wq_sb, qT[g], nc.scalar.copy),
                                (wv_sb, vts, nc.vector.tensor_copy),
                            ):
                                ps = psA.tile(
                                    [128, 512], F32, tag="work", name="ps_kqv"
                                )
                                for kc in range(4):
                                    nc.tensor.matmul(
                                        ps[:],
                                        w_sb[:, kc * 128 : (kc + 1) * 128],
                                        xg[:, kc * 1024 + h * 512 : kc * 1024 + h * 512 + 512],
                                        start=(kc == 0),
                                        stop=(kc == 3),
                                    )
                                copy_eng(dst[:, hs], ps[:])
                        for t in range(8 * g, 8 * g + 8):
                            j = t % 8
                            vps = psA.tile([128, 128], F16, tag="work", name="v_tr")
                            nc.tensor.transpose(
                                vps[:], vts[:, j * 128 : (j + 1) * 128], ident16[:]
                            )
                            # one strided copy scatters both head halves past
                            # the ones columns (cols 64/129 stay 1.0)
                            nc.vector.tensor_copy(
                                v[t][:, 0:130].rearrange("p (g w) -> p g w", g=2)[
                                    :, :, 0:64
                                ],
                                vps[:].rearrange("p (g w) -> p g w", g=2),
                            )
                        # qc=0 attention over this group's key tiles,
                        # software-pipelined: PV(kt-1) issues after
                        # scores(kt) so the PE queue never heads-of-line
                        # blocks on an exp result
                        for kt in range(8 * g, 8 * g + 8):
                            pts = scores_step(0, kt, psS, ptp)
                            if prev_pts is not None:
                                pv_step(kt - 1, prev_pts, pv0_0, pv1_0, n_xt - 1)
                            prev_pts = pts
                    pv_step(n_xt - 1, prev_pts, pv0_0, pv1_0, n_xt - 1)

                # ---- steady state: remaining chunks; each chunk's
                # projection is interleaved into the NEXT chunk's kt loop so
                # its PE/DVE work hides under the exp stream.
                with tc.tile_pool(name="psT", bufs=1, space="PSUM") as psT:
                    prev = drain(0, pv0_0, pv1_0, smp)
                    prev_qc = 0
                    for qc in range(1, n_qc):
                        pv0 = psV.tile([65, 512], F32, tag="pv0", name="pv0")
                        pv1 = psV.tile([65, 512], F32, tag="pv1", name="pv1")
                        prev_pts = None
                        for kt in range(n_xt):
                            pts = scores_step(qc, kt, psS, ptp)
                            if prev_pts is not None:
                                pv_step(kt - 1, prev_pts, pv0, pv1, n_xt - 1)
                            prev_pts = pts
                            if kt % 8 == 7 and kt // 8 < 3:
                                proj_qtile(prev_qc, kt // 8, prev, smp, osp, psT)
                        pv_step(n_xt - 1, prev_pts, pv0, pv1, n_xt - 1)
                        proj_qtile(prev_qc, 3, prev, smp, osp, psT)
                        prev = drain(qc, pv0, pv1, smp)
                        prev_qc = qc
                    for qs in range(4):
                        proj_qtile(prev_qc, qs, prev, smp, osp, psT)
    nc.compile()
    return nc


_CACHE = {}


def _get_nc(tokens=N):
    if tokens not in _CACHE:
        _CACHE[tokens] = build(tokens)
    return _CACHE[tokens]


def _prep_w(w_slice):
    """[512, 128] -> [128, 512] fp16, layout w_[p, kc*128 + j] = w[kc*128 + p, j]."""
    w = np.asarray(w_slice, dtype=np.float16)
    return np.ascontiguousarray(
        w.reshape(4, 128, 128).transpose(1, 0, 2).reshape(128, 512)
    )


def _shard_inputs(x, w_qkv, w_proj):
    in_maps = []
    xt = [
        np.ascontiguousarray(x[b].T.astype(np.float16)) for b in range(x.shape[0])
    ]
    for c in range(8):
        b, hp = divmod(c, 4)
        o = 128 * hp
        in_maps.append(
            {
                "xt": xt[b],
                "wq": _prep_w(w_qkv[:, o : o + 128]),
                "wk": _prep_w(w_qkv[:, 512 + o : 512 + o + 128]),
                "wv": _prep_w(w_qkv[:, 1024 + o : 1024 + o + 128]),
                "wp": np.ascontiguousarray(
                    w_proj[o : o + 128, :], dtype=np.float16
                ),
            }
        )
    return in_maps


def run(x, w_qkv, w_proj, b_proj, trace=False, **kwargs):
    from concourse.bass_utils import run_bass_kernel_spmd

    nc = _get_nc()
    in_maps = _shard_inputs(
        np.asarray(x), np.asarray(w_qkv), np.asarray(w_proj)
    )
    br = run_bass_kernel_spmd(nc, in_maps, list(range(8)), trace=trace, **kwargs)
    parts = [np.asarray(br.results[c]["out"]) for c in range(8)]
    bp = np.asarray(b_proj)
    o0 = parts[0] + parts[1] + parts[2] + parts[3] + bp
    o1 = parts[4] + parts[5] + parts[6] + parts[7] + bp
    return np.stack([o0, o1]).astype(np.float32), br


def kernel(x, w_qkv, w_proj, b_proj):
    result, _ = run(x, w_qkv, w_proj, b_proj, trace=False)
    return result


# revision 54
# speedup vs baseline: 1.0225x; 1.0225x over previous
"""Trainium2 Bass kernel for multi-head self-attention (B=2, N=4096, C=512, H=8).

Sharding: 8 cores = 2 batches x 4 head-pairs. Core c handles batch c//4 and
heads {2*(c%4), 2*(c%4)+1}. Each core computes its two heads' attention over
all 4096 tokens and a partial output projection restricted to its heads' 128
channels; the host sums the 4 partials per batch (the tensor-parallel proj
all-reduce) and adds b_proj.

Host ships x pre-transposed as fp16 xT [512, 4096] and fp16 weights, so the
device prologue is just DMA + qkv matmuls (no PE x-transposes, no DVE casts).
All matmul operands are fp16 (1 cycle/row on the PE vs 2 for f32r).

The softmax exp stream — the steady-state bottleneck — is split across BOTH
transcendental-capable paths by head: head0's exp runs on ScalarE (LUT exp),
head1's on VectorE via a Schraudolph-style fast exp (one fused mult+add
tensor_scalar emitting fp16 bit patterns through a saturating uint16 cast).
Each head's chain PE(scores) -> exp engine -> PE(PV) only ever waits on its
own exp engine. The kt loop is software-pipelined (PV of kt-1 issues after
scores of kt) so the in-order PE queue never head-of-line blocks on an exp.

All P values carry a free 2^-5 scale (folded into the exp bias / fast-exp
constant) so softmax denominators fit fp16; the scale cancels in P/sum and
lets the sums->partition transpose matmuls run in fp16 (K=1 fp32 matmuls
double-pump the PE).

Device dataflow per core (scores never touch DRAM):
  kT = wk^T @ xT           [128, 4096]   (rows 0-63 head0, 64-127 head1)
  qT likewise              [128, 4096]
  v  = transpose(wv^T @ xT) per 128-token tile: [Vh0 | 1 | Vh1 | 1] [128,130]
  per 512-query chunk, per 128-key tile:
    S^T = kT_tile^T @ qT   (two concurrent K=64 row-packed matmuls)
    P^T = 2^-5 exp(SCALE * S^T)  (ScalarE for h0, DVE fast-exp for h1)
    PV += [V|1]^T @ P^T    (PSUM accumulate; row 64 = softmax denominators)
  proj: out_qtile = (outT_h0^T @ wp_h0) * (1/sum_h0) + (outT_h1^T @ wp_h1) * (1/sum_h1)
"""

import math
import os
import sys

if "/opt/trn_rl_repo" not in sys.path:
    sys.path.insert(0, "/opt/trn_rl_repo")

import numpy as np

import concourse.bass as bass
import concourse.mybir as mybir
import concourse.tile as tile
from concourse import bacc
from concourse.masks import make_identity

B, N, C, H = 2, 4096, 512, 8
D = C // H
SCALE = D**-0.5
F32 = mybir.dt.float32
F16 = mybir.dt.float16
# uint16 so the DVE float->unsigned cast saturates negatives to +0.0
# (an out-of-range-low fast-exp result must clamp to P=0, and negative
# int16 bit patterns would decode as fp16 NaN/-huge)
U16 = mybir.dt.uint16

# All P values are scaled by 2^-SHIFT (free via the exp bias) so softmax
# denominators stay well inside fp16 range; the scale cancels in P/sum.
SHIFT = 5
SUMS16 = True

# Schraudolph fast-exp constants for fp16 bit patterns:
#   bits = round(1024 * (log2(e)*SCALE*s + 15 - SHIFT - c))
SCH_A = 1024.0 * SCALE * math.log2(math.e)
SCH_C = float(os.environ.get("ATTN_SCH_C", "0.0579"))
SCH_B = 1024.0 * (15.0 - SHIFT - SCH_C)
EXP_BIAS = -SHIFT * math.log(2.0)

MM_DT_NAME = "f16, exp split ScalarE(h0)/DVE-fast-exp(h1)"


def build(tokens=N, timing=False):
    T = tokens
    n_xt = T // 128  # 128-token tiles (key tiles / v tiles)
    n_g = T // 1024  # 1024-token groups for kT/qT/xT
    n_qc = T // 512  # query chunks

    EXP = mybir.ActivationFunctionType.Exp
    MUL = mybir.AluOpType.mult
    ADD = mybir.AluOpType.add

    nc = bacc.Bacc(None)
    # x arrives host-pre-transposed fp16: xt[c, t] = x[t, c]
    xt = nc.dram_tensor("xt", [C, T], F16, kind="ExternalInput")
    out = nc.dram_tensor("out", [T, C], F32, kind="ExternalOutput")
    # weights arrive host-pre-transposed fp16 into SBUF layout [128, 512]:
    # w*_[p, kc*128 + j] = w[kc*128 + p, j]
    wq = nc.dram_tensor("wq", [128, 512], F16, kind="ExternalInput")
    wk = nc.dram_tensor("wk", [128, 512], F16, kind="ExternalInput")
    wv = nc.dram_tensor("wv", [128, 512], F16, kind="ExternalInput")
    wp = nc.dram_tensor("wp", [128, C], F16, kind="ExternalInput")

    with tile.TileContext(nc) as tc:
        with tc.tile_pool(name="persist", bufs=1) as pp:
            # first x group DMA goes out before everything else (the weight
            # DMAs queue behind it on the sync engine) so the first kq
            # matmul starts as early as possible
            xg0 = pp.tile([128, 4096], F16, tag="xg0", name="xg0")
            nc.sync.dma_start(
                out=xg0[:].rearrange("p (c w) -> p c w", c=4),
                in_=xt[:, 0:1024].rearrange("(c p) w -> p c w", c=4),
            )
            ident = pp.tile([128, 128], F32, tag="ident")
            make_identity(nc, ident[:])
            ident16 = pp.tile([128, 128], F16, tag="ident16")
            nc.vector.tensor_copy(ident16[:], ident[:])
            ebias = pp.tile([128, 1], F32, tag="ebias")
            nc.gpsimd.memset(ebias[:], EXP_BIAS)

            w_sbs = {}
            for wname, wdram in (("wq", wq), ("wk", wk), ("wv", wv), ("wp", wp)):
                w_sb = pp.tile([128, 512], F16, tag=f"{wname}_sb", name=f"{wname}_sb")
                nc.sync.dma_start(out=w_sb[:], in_=wdram[:, :])
                w_sbs[wname] = w_sb
            wq_sb, wk_sb, wv_sb, wp_sb = (
                w_sbs["wq"],
                w_sbs["wk"],
                w_sbs["wv"],
                w_sbs["wp"],
            )

            kT = [
                pp.tile([128, 1024], F16, tag=f"kT{g}", name=f"kT{g}")
                for g in range(n_g)
            ]
            qT = [
                pp.tile([128, 1024], F16, tag=f"qT{g}", name=f"qT{g}")
                for g in range(n_g)
            ]
            v = [
                pp.tile([128, 130], F16, tag=f"v{t}", name=f"v{t}")
                for t in range(n_xt)
            ]
            for t in range(n_xt):
                nc.gpsimd.memset(v[t][:, 64:65], 1.0)
                nc.gpsimd.memset(v[t][:, 129:130], 1.0)
            outT = [
                pp.tile([128, 512], F16, tag=f"outT{s}", name=f"outT{s}")
                for s in range(n_qc)
            ]

            def qt_slice(qc):
                return qT[qc // 2][:, (qc % 2) * 512 : (qc % 2) * 512 + 512]

            def scores_step(qc, kt, psS, ptp):
                # head0's chain runs PE->ScalarE->PE, head1's PE->DVE->PE:
                # each PV matmul depends on exactly one exp engine, so a
                # hiccup on one engine never stalls the other head's chain.
                sc0 = psS.tile([128, 512], F32, tag="sc0", name="sc0")
                sc1 = psS.tile([128, 512], F32, tag="sc1", name="sc1")
                kslc = kT[kt // 8][:, (kt % 8) * 128 : (kt % 8 + 1) * 128]
                qslc = qt_slice(qc)
                nc.tensor.matmul(
                    sc0[:],
                    kslc[0:64, :],
                    qslc[0:64, :],
                    start=True,
                    stop=True,
                    tile_position=(0, 0),
                )
                nc.tensor.matmul(
                    sc1[:],
                    kslc[64:128, :],
                    qslc[64:128, :],
                    start=True,
                    stop=True,
                    tile_position=(64, 0),
                )
                pt0 = ptp.tile([128, 512], U16, tag="pt0", name="pt0")
                pt1 = ptp.tile([128, 512], U16, tag="pt1", name="pt1")
                nc.scalar.activation(
                    pt0[:].bitcast(F16), sc0[:], EXP, bias=ebias[:], scale=SCALE
                )
                # fast exp: fp16 bits = round(SCH_A * s + SCH_B)
                nc.vector.tensor_scalar(pt1[:], sc1[:], SCH_A, SCH_B, MUL, ADD)
                return pt0, pt1

            def pv_step(kt, pts, pv0, pv1, n_last):
                pt0, pt1 = pts
                nc.tensor.matmul(
                    pv0[:],
                    v[kt][:, 0:65],
                    pt0[:].bitcast(F16),
                    start=(kt == 0),
                    stop=(kt == n_last),
                )
                nc.tensor.matmul(
                    pv1[:],
                    v[kt][:, 65:130],
                    pt1[:].bitcast(F16),
                    start=(kt == 0),
                    stop=(kt == n_last),
                )

            def drain(qc, pv0, pv1, smp):
                sums = smp.tile([1, 1024], F16, tag="sums", name="sums")
                # h0 drains on ScalarE, h1 on DVE (keeps both exp engines'
                # side-work symmetric)
                nc.scalar.copy(outT[qc][0:64, :], pv0[0:64, :])
                nc.scalar.copy(sums[0:1, 0:512], pv0[64:65, :])
                nc.vector.tensor_copy(outT[qc][64:128, :], pv1[0:64, :])
                nc.vector.tensor_copy(sums[0:1, 512:1024], pv1[64:65, :])
                return sums

            def proj_qtile(qc, qs, sums, smp, osp, psT):
                i = qc * 4 + qs
                ta = psT.tile([128, 512], F32, tag="ta", name="ta")
                tb = psT.tile([128, 512], F32, tag="tb", name="tb")
                # denominators -> partition layout via K=1 fp16 matmuls
                nc.tensor.matmul(
                    ta[:, 0:1],
                    sums[0:1, qs * 128 : (qs + 1) * 128],
                    ident16[0:1, 0:1],
                    start=True,
                    stop=True,
                )
                nc.tensor.matmul(
                    ta[:, 1:2],
                    sums[0:1, 512 + qs * 128 : 512 + (qs + 1) * 128],
                    ident16[0:1, 0:1],
                    start=True,
                    stop=True,
                )
                rc = smp.tile([128, 2], F32, tag="recip", name="rc")
                nc.vector.reciprocal(rc[:], ta[:, 0:2])
                oslc = outT[qc][:, qs * 128 : (qs + 1) * 128]
                nc.tensor.matmul(
                    ta[:],
                    oslc[0:64, :],
                    wp_sb[0:64, :],
                    start=True,
                    stop=True,
                    tile_position=(0, 0),
                )
                nc.tensor.matmul(
                    tb[:],
                    oslc[64:128, :],
                    wp_sb[64:128, :],
                    start=True,
                    stop=True,
                    tile_position=(64, 0),
                )
                t0 = osp.tile([128, 512], F32, tag="t0", name="t0")
                nc.scalar.activation(
                    t0[:], ta[:], mybir.ActivationFunctionType.Copy,
                    bias=0.0, scale=rc[:, 0:1],
                )
                ob = osp.tile([128, 512], F32, tag="ob", name="ob")
                nc.vector.scalar_tensor_tensor(
                    ob[:], tb[:], rc[:, 1:2], t0[:], op0=MUL, op1=ADD
                )
                nc.sync.dma_start(out=out[i * 128 : (i + 1) * 128, :], in_=ob[:])

            with tc.tile_pool(name="ptp", bufs=6) as ptp, tc.tile_pool(
                name="smp", bufs=2
            ) as smp, tc.tile_pool(name="osp", bufs=2) as osp, tc.tile_pool(
                name="psS", bufs=2, space="PSUM"
            ) as psS, tc.tile_pool(name="psV", bufs=1, space="PSUM") as psV:
                pv0_0 = psV.tile([65, 512], F32, tag="pv0", name="pv0")
                pv1_0 = psV.tile([65, 512], F32, tag="pv1", name="pv1")

                # ---- prologue: produce kT/qT/v per 1024-token group, with
                # qc=0's attention interleaved so the exp engines start early
                with tc.tile_pool(name="ldp", bufs=2) as ldp, tc.tile_pool(
                    name="psA", bufs=1, space="PSUM"
                ) as psA:
                    prev_pts = None
                    for g in range(n_g):
                        if g == 0:
                            xg = xg0
                        else:
                            xg = ldp.tile([128, 4096], F16, tag="xload", name="xg")
                            nc.sync.dma_start(
                                out=xg[:].rearrange("p (c w) -> p c w", c=4),
                                in_=xt[:, g * 1024 : (g + 1) * 1024].rearrange(
                                    "(c p) w -> p c w", c=4
                                ),
                            )
                        vts = ldp.tile([128, 1024], F16, tag="vts", name="vts")
                        for h in range(2):
                            hs = slice(h * 512, h * 512 + 512)
                            for w_sb, dst, copy_eng in (
                                (wk_sb, kT[g], nc.scalar.copy),
                                (wq_sb, qT[g], nc.scalar.copy),
                                (wv_sb, vts, nc.vector.tensor_copy),
                            ):
                                ps = psA.tile(
                                    [128, 512], F32, tag="work", name="ps_kqv"
                                )
                                for kc in range(4):
                                    nc.tensor.matmul(
                                        ps[:],
                                        w_sb[:, kc * 128 : (kc + 1) * 128],
                                        xg[:, kc * 1024 + h * 512 : kc * 1024 + h * 512 + 512],
                                        start=(kc == 0),
                                        stop=(kc == 3),
                                    )
                                copy_eng(dst[:, hs], ps[:])
                        for t in range(8 * g, 8 * g + 8):
                            j = t % 8
                            vps = psA.tile([128, 128], F16, tag="work", name="v_tr")
                            nc.tensor.transpose(
                                vps[:], vts[:, j * 128 : (j + 1) * 128], ident16[:]
                            )
                            # one strided copy scatters both head halves past
                            # the ones columns (cols 64/129 stay 1.0)
                            nc.vector.tensor_copy(
                                v[t][:, 0:130].rearrange("p (g w) -> p g w", g=2)[
                                    :, :, 0:64
                                ],
                                vps[:].rearrange("p (g w) -> p g w", g=2),
                            )
                        # qc=0 attention over this group's key tiles,
                        # software-pipelined: PV(kt-1) issues after
                        # scores(kt) so the PE queue never heads-of-line
                        # blocks on an exp result
                        for kt in range(8 * g, 8 * g + 8):
                            pts = scores_step(0, kt, psS, ptp)
                            if prev_pts is not None:
                                pv_step(kt - 1, prev_pts, pv0_0, pv1_0, n_xt - 1)
                            prev_pts = pts
                    pv_step(n_xt - 1, prev_pts, pv0_0, pv1_0, n_xt - 1)

                # ---- steady state: remaining chunks; each chunk's
                # projection is interleaved into the NEXT chunk's kt loop so
                # its PE/DVE work hides under the exp stream.
                with tc.tile_pool(name="psT", bufs=1, space="PSUM") as psT:
                    prev = drain(0, pv0_0, pv1_0, smp)
                    prev_qc = 0
                    for qc in range(1, n_qc):
                        pv0 = psV.tile([65, 512], F32, tag="pv0", name="pv0")
                        pv1 = psV.tile([65, 512], F32, tag="pv1", name="pv1")
                        prev_pts = None
                        for kt in range(n_xt):
                            pts = scores_step(qc, kt, psS, ptp)
                            if prev_pts is not None:
                                pv_step(kt - 1, prev_pts, pv0, pv1, n_xt - 1)
                            prev_pts = pts
                            if kt % 8 == 7 and kt // 8 < 3:
                                proj_qtile(prev_qc, kt // 8, prev, smp, osp, psT)
                        pv_step(n_xt - 1, prev_pts, pv0, pv1, n_xt - 1)
                        proj_qtile(prev_qc, 3, prev, smp, osp, psT)
                        prev = drain(qc, pv0, pv1, smp)
                        prev_qc = qc
                    for qs in range(4):
                        proj_qtile(prev_qc, qs, prev, smp, osp, psT)
    nc.compile()
    return nc


_CACHE = {}


def _get_nc(tokens=N):
    if tokens not in _CACHE:
        _CACHE[tokens] = build(tokens)
    return _CACHE[tokens]


def _prep_w(w_slice):
    """[512, 128] -> [128, 512] fp16, layout w_[p, kc*128 + j] = w[kc*128 + p, j]."""
    w = np.asarray(w_slice, dtype=np.float16)
    return np.ascontiguousarray(
        w.reshape(4, 128, 128).transpose(1, 0, 2).reshape(128, 512)
    )


def _shard_inputs(x, w_qkv, w_proj):
    in_maps = []
    xt = [
        np.ascontiguousarray(x[b].T.astype(np.float16)) for b in range(x.shape[0])
    ]
    for c in range(8):
        b, hp = divmod(c, 4)
        o = 128 * hp
        in_maps.append(
            {
                "xt": xt[b],
                "wq": _prep_w(w_qkv[:, o : o + 128]),
                "wk": _prep_w(w_qkv[:, 512 + o : 512 + o + 128]),
                "wv": _prep_w(w_qkv[:, 1024 + o : 1024 + o + 128]),
                "wp": np.ascontiguousarray(
                    w_proj[o : o + 128, :], dtype=np.float16
                ),
            }
        )
    return in_maps


def run(x, w_qkv, w_proj, b_proj, trace=False, **kwargs):
    from concourse.bass_utils import run_bass_kernel_spmd

    nc = _get_nc()
    in_maps = _shard_inputs(
        np.asarray(x), np.asarray(w_qkv), np.asarray(w_proj)
    )
    br = run_bass_kernel_spmd(nc, in_maps, list(range(8)), trace=trace, **kwargs)
    parts = [np.asarray(br.results[c]["out"]) for c in range(8)]
    bp = np.asarray(b_proj)
    o0 = parts[0] + parts[1] + parts[2] + parts[3] + bp
    o1 = parts[4] + parts[5] + parts[6] + parts[7] + bp
    return np.stack([o0, o1]).astype(np.float32), br


def kernel(x, w_qkv, w_proj, b_proj):
    result, _ = run(x, w_qkv, w_proj, b_proj, trace=False)
    return result


# revision 55
# speedup vs baseline: 1.0289x; 1.0063x over previous
"""Trainium2 Bass kernel for multi-head self-attention (B=2, N=4096, C=512, H=8).

Sharding: 8 cores = 2 batches x 4 head-pairs. Core c handles batch c//4 and
heads {2*(c%4), 2*(c%4)+1}. Each core computes its two heads' attention over
all 4096 tokens and a partial output projection restricted to its heads' 128
channels; the host sums the 4 partials per batch (the tensor-parallel proj
all-reduce) and adds b_proj.

Host ships x pre-transposed as fp16 xT [512, 4096] and fp16 weights, so the
device prologue is just DMA + qkv matmuls (no PE x-transposes, no DVE casts).
All matmul operands are fp16 (1 cycle/row on the PE vs 2 for f32r).

The softmax exp stream — the steady-state bottleneck — is split across BOTH
transcendental-capable paths by head: head0's exp runs on ScalarE (LUT exp),
head1's on VectorE via a Schraudolph-style fast exp (one fused mult+add
tensor_scalar emitting fp16 bit patterns through a saturating uint16 cast).
Each head's chain PE(scores) -> exp engine -> PE(PV) only ever waits on its
own exp engine. The kt loop is software-pipelined (PV of kt-1 issues after
scores of kt) so the in-order PE queue never head-of-line blocks on an exp.

All P values carry a free 2^-5 scale (folded into the exp bias / fast-exp
constant) so softmax denominators fit fp16; the scale cancels in P/sum and
lets the sums->partition transpose matmuls run in fp16 (K=1 fp32 matmuls
double-pump the PE).

Device dataflow per core (scores never touch DRAM):
  kT = wk^T @ xT           [128, 4096]   (rows 0-63 head0, 64-127 head1)
  qT likewise              [128, 4096]
  v  = transpose(wv^T @ xT) per 128-token tile: [Vh0 | 1 | Vh1 | 1] [128,130]
  per 512-query chunk, per 128-key tile:
    S^T = kT_tile^T @ qT   (two concurrent K=64 row-packed matmuls)
    P^T = 2^-5 exp(SCALE * S^T)  (ScalarE for h0, DVE fast-exp for h1)
    PV += [V|1]^T @ P^T    (PSUM accumulate; row 64 = softmax denominators)
  proj: out_qtile = (outT_h0^T @ wp_h0) * (1/sum_h0) + (outT_h1^T @ wp_h1) * (1/sum_h1)
"""

import math
import os
import sys

if "/opt/trn_rl_repo" not in sys.path:
    sys.path.insert(0, "/opt/trn_rl_repo")

import numpy as np

import concourse.bass as bass
import concourse.mybir as mybir
import concourse.tile as tile
from concourse import bacc
from concourse.masks import make_identity

B, N, C, H = 2, 4096, 512, 8
D = C // H
SCALE = D**-0.5
F32 = mybir.dt.float32
F16 = mybir.dt.float16
# uint16 so the DVE float->unsigned cast saturates negatives to +0.0
# (an out-of-range-low fast-exp result must clamp to P=0, and negative
# int16 bit patterns would decode as fp16 NaN/-huge)
U16 = mybir.dt.uint16

# All P values are scaled by 2^-SHIFT (free via the exp bias) so softmax
# denominators stay well inside fp16 range; the scale cancels in P/sum.
SHIFT = 5
SUMS16 = True

# Schraudolph fast-exp constants for fp16 bit patterns:
#   bits = round(1024 * (log2(e)*SCALE*s + 15 - SHIFT - c))
SCH_A = 1024.0 * SCALE * math.log2(math.e)
SCH_C = float(os.environ.get("ATTN_SCH_C", "0.0579"))
SCH_B = 1024.0 * (15.0 - SHIFT - SCH_C)
EXP_BIAS = -SHIFT * math.log(2.0)

MM_DT_NAME = "f16, exp split ScalarE(h0)/DVE-fast-exp(h1)"


def build(tokens=N, timing=False):
    T = tokens
    n_xt = T // 128  # 128-token tiles (key tiles / v tiles)
    n_g = T // 1024  # 1024-token groups for kT/qT/xT
    n_qc = T // 512  # query chunks

    EXP = mybir.ActivationFunctionType.Exp
    MUL = mybir.AluOpType.mult
    ADD = mybir.AluOpType.add

    nc = bacc.Bacc(None)
    # x arrives host-pre-transposed fp16: xt[c, t] = x[t, c]
    xt = nc.dram_tensor("xt", [C, T], F16, kind="ExternalInput")
    out = nc.dram_tensor("out", [T, C], F32, kind="ExternalOutput")
    # weights arrive host-pre-transposed fp16 into SBUF layout [128, 512]:
    # w*_[p, kc*128 + j] = w[kc*128 + p, j]
    wq = nc.dram_tensor("wq", [128, 512], F16, kind="ExternalInput")
    wk = nc.dram_tensor("wk", [128, 512], F16, kind="ExternalInput")
    wv = nc.dram_tensor("wv", [128, 512], F16, kind="ExternalInput")
    wp = nc.dram_tensor("wp", [128, C], F16, kind="ExternalInput")

    with tile.TileContext(nc) as tc:
        with tc.tile_pool(name="persist", bufs=1) as pp:
            # first x group DMA goes out before everything else (the weight
            # DMAs queue behind it on the sync engine) so the first kq
            # matmul starts as early as possible
            xg0 = pp.tile([128, 4096], F16, tag="xg0", name="xg0")
            nc.sync.dma_start(
                out=xg0[:].rearrange("p (c w) -> p c w", c=4),
                in_=xt[:, 0:1024].rearrange("(c p) w -> p c w", c=4),
            )
            ident = pp.tile([128, 128], F32, tag="ident")
            make_identity(nc, ident[:])
            ident16 = pp.tile([128, 128], F16, tag="ident16")
            nc.vector.tensor_copy(ident16[:], ident[:])
            ebias = pp.tile([128, 1], F32, tag="ebias")
            nc.gpsimd.memset(ebias[:], EXP_BIAS)

            w_sbs = {}
            for wname, wdram in (("wq", wq), ("wk", wk), ("wv", wv), ("wp", wp)):
                w_sb = pp.tile([128, 512], F16, tag=f"{wname}_sb", name=f"{wname}_sb")
                nc.sync.dma_start(out=w_sb[:], in_=wdram[:, :])
                w_sbs[wname] = w_sb
            wq_sb, wk_sb, wv_sb, wp_sb = (
                w_sbs["wq"],
                w_sbs["wk"],
                w_sbs["wv"],
                w_sbs["wp"],
            )

            kT = [
                pp.tile([128, 1024], F16, tag=f"kT{g}", name=f"kT{g}")
                for g in range(n_g)
            ]
            qT = [
                pp.tile([128, 1024], F16, tag=f"qT{g}", name=f"qT{g}")
                for g in range(n_g)
            ]
            v = [
                pp.tile([128, 130], F16, tag=f"v{t}", name=f"v{t}")
                for t in range(n_xt)
            ]
            for t in range(n_xt):
                nc.gpsimd.memset(v[t][:, 64:65], 1.0)
                nc.gpsimd.memset(v[t][:, 129:130], 1.0)
            outT = [
                pp.tile([128, 512], F16, tag=f"outT{s}", name=f"outT{s}")
                for s in range(n_qc)
            ]

            def qt_slice(qc):
                return qT[qc // 2][:, (qc % 2) * 512 : (qc % 2) * 512 + 512]

            def scores_step(qc, kt, psS, ptp):
                # head0's chain runs PE->ScalarE->PE, head1's PE->DVE->PE:
                # each PV matmul depends on exactly one exp engine, so a
                # hiccup on one engine never stalls the other head's chain.
                sc0 = psS.tile([128, 512], F32, tag="sc0", name="sc0")
                sc1 = psS.tile([128, 512], F32, tag="sc1", name="sc1")
                kslc = kT[kt // 8][:, (kt % 8) * 128 : (kt % 8 + 1) * 128]
                qslc = qt_slice(qc)
                nc.tensor.matmul(
                    sc0[:],
                    kslc[0:64, :],
                    qslc[0:64, :],
                    start=True,
                    stop=True,
                    tile_position=(0, 0),
                )
                nc.tensor.matmul(
                    sc1[:],
                    kslc[64:128, :],
                    qslc[64:128, :],
                    start=True,
                    stop=True,
                    tile_position=(64, 0),
                )
                pt0 = ptp.tile([128, 512], U16, tag="pt0", name="pt0")
                pt1 = ptp.tile([128, 512], U16, tag="pt1", name="pt1")
                nc.scalar.activation(
                    pt0[:].bitcast(F16), sc0[:], EXP, bias=ebias[:], scale=SCALE
                )
                # fast exp: fp16 bits = round(SCH_A * s + SCH_B)
                nc.vector.tensor_scalar(pt1[:], sc1[:], SCH_A, SCH_B, MUL, ADD)
                return pt0, pt1

            def pv_step(kt, pts, pv0, pv1, n_last):
                pt0, pt1 = pts
                nc.tensor.matmul(
                    pv0[:],
                    v[kt][:, 0:65],
                    pt0[:].bitcast(F16),
                    start=(kt == 0),
                    stop=(kt == n_last),
                )
                nc.tensor.matmul(
                    pv1[:],
                    v[kt][:, 65:130],
                    pt1[:].bitcast(F16),
                    start=(kt == 0),
                    stop=(kt == n_last),
                )

            def drain(qc, pv0, pv1, smp):
                sums = smp.tile([1, 1024], F16, tag="sums", name="sums")
                # h0 drains on ScalarE, h1 on DVE (keeps both exp engines'
                # side-work symmetric)
                nc.scalar.copy(outT[qc][0:64, :], pv0[0:64, :])
                nc.scalar.copy(sums[0:1, 0:512], pv0[64:65, :])
                nc.vector.tensor_copy(outT[qc][64:128, :], pv1[0:64, :])
                nc.vector.tensor_copy(sums[0:1, 512:1024], pv1[64:65, :])
                return sums

            def proj_qtile(qc, qs, sums, smp, osp, psT):
                i = qc * 4 + qs
                ta = psT.tile([128, 512], F32, tag="ta", name="ta")
                tb = psT.tile([128, 512], F32, tag="tb", name="tb")
                # denominators -> partition layout via K=1 fp16 matmuls
                nc.tensor.matmul(
                    ta[:, 0:1],
                    sums[0:1, qs * 128 : (qs + 1) * 128],
                    ident16[0:1, 0:1],
                    start=True,
                    stop=True,
                )
                nc.tensor.matmul(
                    ta[:, 1:2],
                    sums[0:1, 512 + qs * 128 : 512 + (qs + 1) * 128],
                    ident16[0:1, 0:1],
                    start=True,
                    stop=True,
                )
                rc = smp.tile([128, 2], F32, tag="recip", name="rc")
                nc.vector.reciprocal(rc[:], ta[:, 0:2])
                oslc = outT[qc][:, qs * 128 : (qs + 1) * 128]
                nc.tensor.matmul(
                    ta[:],
                    oslc[0:64, :],
                    wp_sb[0:64, :],
                    start=True,
                    stop=True,
                    tile_position=(0, 0),
                )
                nc.tensor.matmul(
                    tb[:],
                    oslc[64:128, :],
                    wp_sb[64:128, :],
                    start=True,
                    stop=True,
                    tile_position=(64, 0),
                )
                t0 = osp.tile([128, 512], F32, tag="t0", name="t0")
                nc.scalar.activation(
                    t0[:], ta[:], mybir.ActivationFunctionType.Copy,
                    bias=0.0, scale=rc[:, 0:1],
                )
                ob = osp.tile([128, 512], F32, tag="ob", name="ob")
                nc.vector.scalar_tensor_tensor(
                    ob[:], tb[:], rc[:, 1:2], t0[:], op0=MUL, op1=ADD
                )
                nc.sync.dma_start(out=out[i * 128 : (i + 1) * 128, :], in_=ob[:])

            with tc.tile_pool(name="ptp", bufs=6) as ptp, tc.tile_pool(
                name="smp", bufs=2
            ) as smp, tc.tile_pool(name="osp", bufs=2) as osp, tc.tile_pool(
                name="psS", bufs=2, space="PSUM"
            ) as psS, tc.tile_pool(name="psV", bufs=1, space="PSUM") as psV:
                pv0_0 = psV.tile([65, 512], F32, tag="pv0", name="pv0")
                pv1_0 = psV.tile([65, 512], F32, tag="pv1", name="pv1")

                # ---- prologue: produce kT/qT/v per 1024-token group, with
                # qc=0's attention interleaved so the exp engines start early
                with tc.tile_pool(name="ldp", bufs=2) as ldp, tc.tile_pool(
                    name="psA", bufs=1, space="PSUM"
                ) as psA:
                    prev_pts = None
                    for g in range(n_g):
                        if g == 0:
                            xg = xg0
                        else:
                            xg = ldp.tile([128, 4096], F16, tag="xload", name="xg")
                            nc.sync.dma_start(
                                out=xg[:].rearrange("p (c w) -> p c w", c=4),
                                in_=xt[:, g * 1024 : (g + 1) * 1024].rearrange(
                                    "(c p) w -> p c w", c=4
                                ),
                            )
                        vts = ldp.tile([128, 1024], F16, tag="vts", name="vts")
                        for h in range(2):
                            hs = slice(h * 512, h * 512 + 512)
                            for w_sb, dst, copy_eng in (
                                (wk_sb, kT[g], nc.scalar.copy),
                                (wq_sb, qT[g], nc.scalar.copy),
                                (wv_sb, vts, nc.vector.tensor_copy),
                            ):
                                ps = psA.tile(
                                    [128, 512], F32, tag="work", name="ps_kqv"
                                )
                                for kc in range(4):
                                    nc.tensor.matmul(
                                        ps[:],
                                        w_sb[:, kc * 128 : (kc + 1) * 128],
                                        xg[:, kc * 1024 + h * 512 : kc * 1024 + h * 512 + 512],
                                        start=(kc == 0),
                                        stop=(kc == 3),
                                    )
                                copy_eng(dst[:, hs], ps[:])
                        for t in range(8 * g, 8 * g + 8):
                            j = t % 8
                            vps = psA.tile([128, 128], F16, tag="work", name="v_tr")
                            nc.tensor.transpose(
                                vps[:], vts[:, j * 128 : (j + 1) * 128], ident16[:]
                            )
                            # one strided copy scatters both head halves past
                            # the ones columns (cols 64/129 stay 1.0)
                            nc.vector.tensor_copy(
                                v[t][:, 0:130].rearrange("p (g w) -> p g w", g=2)[
                                    :, :, 0:64
                                ],
                                vps[:].rearrange("p (g w) -> p g w", g=2),
                            )
                        # qc=0 attention over this group's key tiles,
                        # software-pipelined: PV(kt-1) issues after
                        # scores(kt) so the PE queue never heads-of-line
                        # blocks on an exp result
                        for kt in range(8 * g, 8 * g + 8):
                            pts = scores_step(0, kt, psS, ptp)
                            if prev_pts is not None:
                                pv_step(kt - 1, prev_pts, pv0_0, pv1_0, n_xt - 1)
                            prev_pts = pts
                    pv_step(n_xt - 1, prev_pts, pv0_0, pv1_0, n_xt - 1)

                # ---- steady state: remaining chunks; each chunk's
                # projection is interleaved into the NEXT chunk's kt loop so
                # its PE/DVE work hides under the exp stream.
                with tc.tile_pool(name="psT", bufs=1, space="PSUM") as psT:
                    prev = drain(0, pv0_0, pv1_0, smp)
                    prev_qc = 0
                    for qc in range(1, n_qc):
                        pv0 = psV.tile([65, 512], F32, tag="pv0", name="pv0")
                        pv1 = psV.tile([65, 512], F32, tag="pv1", name="pv1")
                        prev_pts = None
                        for kt in range(n_xt):
                            pts = scores_step(qc, kt, psS, ptp)
                            if prev_pts is not None:
                                pv_step(kt - 1, prev_pts, pv0, pv1, n_xt - 1)
                            prev_pts = pts
                            if kt % 8 == 7 and kt // 8 < 3:
                                proj_qtile(prev_qc, kt // 8, prev, smp, osp, psT)
                        pv_step(n_xt - 1, prev_pts, pv0, pv1, n_xt - 1)
                        proj_qtile(prev_qc, 3, prev, smp, osp, psT)
                        prev = drain(qc, pv0, pv1, smp)
                        prev_qc = qc
                    # tail: hoist all denominator transposes + one recip so
                    # the 16 proj matmuls stream without per-qtile stalls
                    dn = psT.tile([128, 512], F32, tag="tb", name="dn")
                    for qs in range(4):
                        for h in range(2):
                            nc.tensor.matmul(
                                dn[:, 2 * qs + h : 2 * qs + h + 1],
                                prev[0:1, 512 * h + qs * 128 : 512 * h + (qs + 1) * 128],
                                ident16[0:1, 0:1],
                                start=True,
                                stop=True,
                            )
                    rca = smp.tile([128, 8], F32, tag="rcall", name="rca")
                    nc.vector.reciprocal(rca[:], dn[:, 0:8])
                    for qs in range(4):
                        i = prev_qc * 4 + qs
                        ta = psT.tile([128, 512], F32, tag="ta", name="ta")
                        tb = psT.tile([128, 512], F32, tag="tb", name="tb")
                        oslc = outT[prev_qc][:, qs * 128 : (qs + 1) * 128]
                        nc.tensor.matmul(
                            ta[:],
                            oslc[0:64, :],
                            wp_sb[0:64, :],
                            start=True,
                            stop=True,
                            tile_position=(0, 0),
                        )
                        nc.tensor.matmul(
                            tb[:],
                            oslc[64:128, :],
                            wp_sb[64:128, :],
                            start=True,
                            stop=True,
                            tile_position=(64, 0),
                        )
                        t0 = osp.tile([128, 512], F32, tag="t0", name="t0")
                        nc.scalar.activation(
                            t0[:], ta[:], mybir.ActivationFunctionType.Copy,
                            bias=0.0, scale=rca[:, 2 * qs : 2 * qs + 1],
                        )
                        ob = osp.tile([128, 512], F32, tag="ob", name="ob")
                        nc.vector.scalar_tensor_tensor(
                            ob[:], tb[:], rca[:, 2 * qs + 1 : 2 * qs + 2],
                            t0[:], op0=MUL, op1=ADD
                        )
                        nc.sync.dma_start(
                            out=out[i * 128 : (i + 1) * 128, :], in_=ob[:]
                        )
    nc.compile()
    return nc


_CACHE = {}


def _get_nc(tokens=N):
    if tokens not in _CACHE:
        _CACHE[tokens] = build(tokens)
    return _CACHE[tokens]


def _prep_w(w_slice):
    """[512, 128] -> [128, 512] fp16, layout w_[p, kc*128 + j] = w[kc*128 + p, j]."""
    w = np.asarray(w_slice, dtype=np.float16)
    return np.ascontiguousarray(
        w.reshape(4, 128, 128).transpose(1, 0, 2).reshape(128, 512)
    )


def _shard_inputs(x, w_qkv, w_proj):
    in_maps = []
    xt = [
        np.ascontiguousarray(x[b].T.astype(np.float16)) for b in range(x.shape[0])
    ]
    for c in range(8):
        b, hp = divmod(c, 4)
        o = 128 * hp
        in_maps.append(
            {
                "xt": xt[b],
                "wq": _prep_w(w_qkv[:, o : o + 128]),
                "wk": _prep_w(w_qkv[:, 512 + o : 512 + o + 128]),
                "wv": _prep_w(w_qkv[:, 1024 + o : 1024 + o + 128]),
                "wp": np.ascontiguousarray(
                    w_proj[o : o + 128, :], dtype=np.float16
                ),
            }
        )
    return in_maps


def run(x, w_qkv, w_proj, b_proj, trace=False, **kwargs):
    from concourse.bass_utils import run_bass_kernel_spmd

    nc = _get_nc()
    in_maps = _shard_inputs(
        np.asarray(x), np.asarray(w_qkv), np.asarray(w_proj)
    )
    br = run_bass_kernel_spmd(nc, in_maps, list(range(8)), trace=trace, **kwargs)
    parts = [np.asarray(br.results[c]["out"]) for c in range(8)]
    bp = np.asarray(b_proj)
    o0 = parts[0] + parts[1] + parts[2] + parts[3] + bp
    o1 = parts[4] + parts[5] + parts[6] + parts[7] + bp
    return np.stack([o0, o1]).astype(np.float32), br


def kernel(x, w_qkv, w_proj, b_proj):
    result, _ = run(x, w_qkv, w_proj, b_proj, trace=False)
    return result


# revision 56
# speedup vs baseline: 1.0299x; 1.0009x over previous
"""Trainium2 Bass kernel for multi-head self-attention (B=2, N=4096, C=512, H=8).

Sharding: 8 cores = 2 batches x 4 head-pairs. Core c handles batch c//4 and
heads {2*(c%4), 2*(c%4)+1}. Each core computes its two heads' attention over
all 4096 tokens and a partial output projection restricted to its heads' 128
channels; the host sums the 4 partials per batch (the tensor-parallel proj
all-reduce) and adds b_proj.

Host ships x pre-transposed as fp16 xT [512, 4096] and fp16 weights, so the
device prologue is just DMA + qkv matmuls (no PE x-transposes, no DVE casts).
All matmul operands are fp16 (1 cycle/row on the PE vs 2 for f32r).

The softmax exp stream — the steady-state bottleneck — is split across BOTH
transcendental-capable paths by head: head0's exp runs on ScalarE (LUT exp),
head1's on VectorE via a Schraudolph-style fast exp (one fused mult+add
tensor_scalar emitting fp16 bit patterns through a saturating uint16 cast).
Each head's chain PE(scores) -> exp engine -> PE(PV) only ever waits on its
own exp engine. The kt loop is software-pipelined (PV of kt-1 issues after
scores of kt) so the in-order PE queue never head-of-line blocks on an exp.

All P values carry a free 2^-5 scale (folded into the exp bias / fast-exp
constant) so softmax denominators fit fp16; the scale cancels in P/sum and
lets the sums->partition transpose matmuls run in fp16 (K=1 fp32 matmuls
double-pump the PE).

Device dataflow per core (scores never touch DRAM):
  kT = wk^T @ xT           [128, 4096]   (rows 0-63 head0, 64-127 head1)
  qT likewise              [128, 4096]
  v  = transpose(wv^T @ xT) per 128-token tile: [Vh0 | 1 | Vh1 | 1] [128,130]
  per 512-query chunk, per 128-key tile:
    S^T = kT_tile^T @ qT   (two concurrent K=64 row-packed matmuls)
    P^T = 2^-5 exp(SCALE * S^T)  (ScalarE for h0, DVE fast-exp for h1)
    PV += [V|1]^T @ P^T    (PSUM accumulate; row 64 = softmax denominators)
  proj: out_qtile = (outT_h0^T @ wp_h0) * (1/sum_h0) + (outT_h1^T @ wp_h1) * (1/sum_h1)
"""

import math
import os
import sys

if "/opt/trn_rl_repo" not in sys.path:
    sys.path.insert(0, "/opt/trn_rl_repo")

import numpy as np

import concourse.bass as bass
import concourse.mybir as mybir
import concourse.tile as tile
from concourse import bacc
from concourse.masks import make_identity

B, N, C, H = 2, 4096, 512, 8
D = C // H
SCALE = D**-0.5
F32 = mybir.dt.float32
F16 = mybir.dt.float16
# uint16 so the DVE float->unsigned cast saturates negatives to +0.0
# (an out-of-range-low fast-exp result must clamp to P=0, and negative
# int16 bit patterns would decode as fp16 NaN/-huge)
U16 = mybir.dt.uint16

# All P values are scaled by 2^-SHIFT (free via the exp bias) so softmax
# denominators stay well inside fp16 range; the scale cancels in P/sum.
SHIFT = 5
SUMS16 = True

# Schraudolph fast-exp constants for fp16 bit patterns:
#   bits = round(1024 * (log2(e)*SCALE*s + 15 - SHIFT - c))
SCH_A = 1024.0 * SCALE * math.log2(math.e)
SCH_C = float(os.environ.get("ATTN_SCH_C", "0.0579"))
SCH_B = 1024.0 * (15.0 - SHIFT - SCH_C)
EXP_BIAS = -SHIFT * math.log(2.0)

MM_DT_NAME = "f16, exp split ScalarE(h0)/DVE-fast-exp(h1)"


def build(tokens=N, timing=False):
    T = tokens
    n_xt = T // 128  # 128-token tiles (key tiles / v tiles)
    n_g = T // 1024  # 1024-token groups for kT/qT/xT
    n_qc = T // 512  # query chunks

    EXP = mybir.ActivationFunctionType.Exp
    MUL = mybir.AluOpType.mult
    ADD = mybir.AluOpType.add

    nc = bacc.Bacc(None)
    # x arrives host-pre-transposed fp16: xt[c, t] = x[t, c]
    xt = nc.dram_tensor("xt", [C, T], F16, kind="ExternalInput")
    out = nc.dram_tensor("out", [T, C], F32, kind="ExternalOutput")
    # weights arrive host-pre-transposed fp16 into SBUF layout [128, 512]:
    # w*_[p, kc*128 + j] = w[kc*128 + p, j]
    wq = nc.dram_tensor("wq", [128, 512], F16, kind="ExternalInput")
    wk = nc.dram_tensor("wk", [128, 512], F16, kind="ExternalInput")
    wv = nc.dram_tensor("wv", [128, 512], F16, kind="ExternalInput")
    wp = nc.dram_tensor("wp", [128, C], F16, kind="ExternalInput")

    with tile.TileContext(nc) as tc:
        with tc.tile_pool(name="persist", bufs=1) as pp:
            # first x group DMA goes out before everything else (the weight
            # DMAs queue behind it on the sync engine) so the first kq
            # matmul starts as early as possible
            xg0 = pp.tile([128, 4096], F16, tag="xg0", name="xg0")
            # per-chunk transfers: the first kq matmul only needs chunk 0,
            # so it starts ~2us before the full group would land
            for c in range(4):
                nc.sync.dma_start(
                    out=xg0[:, c * 1024 : (c + 1) * 1024],
                    in_=xt[c * 128 : (c + 1) * 128, 0:1024],
                )
            ident = pp.tile([128, 128], F32, tag="ident")
            make_identity(nc, ident[:])
            ident16 = pp.tile([128, 128], F16, tag="ident16")
            nc.vector.tensor_copy(ident16[:], ident[:])
            ebias = pp.tile([128, 1], F32, tag="ebias")
            nc.gpsimd.memset(ebias[:], EXP_BIAS)

            w_sbs = {}
            for wname, wdram in (("wq", wq), ("wk", wk), ("wv", wv), ("wp", wp)):
                w_sb = pp.tile([128, 512], F16, tag=f"{wname}_sb", name=f"{wname}_sb")
                nc.sync.dma_start(out=w_sb[:], in_=wdram[:, :])
                w_sbs[wname] = w_sb
            wq_sb, wk_sb, wv_sb, wp_sb = (
                w_sbs["wq"],
                w_sbs["wk"],
                w_sbs["wv"],
                w_sbs["wp"],
            )

            kT = [
                pp.tile([128, 1024], F16, tag=f"kT{g}", name=f"kT{g}")
                for g in range(n_g)
            ]
            qT = [
                pp.tile([128, 1024], F16, tag=f"qT{g}", name=f"qT{g}")
                for g in range(n_g)
            ]
            v = [
                pp.tile([128, 130], F16, tag=f"v{t}", name=f"v{t}")
                for t in range(n_xt)
            ]
            for t in range(n_xt):
                nc.gpsimd.memset(v[t][:, 64:65], 1.0)
                nc.gpsimd.memset(v[t][:, 129:130], 1.0)
            outT = [
                pp.tile([128, 512], F16, tag=f"outT{s}", name=f"outT{s}")
                for s in range(n_qc)
            ]

            def qt_slice(qc):
                return qT[qc // 2][:, (qc % 2) * 512 : (qc % 2) * 512 + 512]

            def scores_step(qc, kt, psS, ptp):
                # head0's chain runs PE->ScalarE->PE, head1's PE->DVE->PE:
                # each PV matmul depends on exactly one exp engine, so a
                # hiccup on one engine never stalls the other head's chain.
                sc0 = psS.tile([128, 512], F32, tag="sc0", name="sc0")
                sc1 = psS.tile([128, 512], F32, tag="sc1", name="sc1")
                kslc = kT[kt // 8][:, (kt % 8) * 128 : (kt % 8 + 1) * 128]
                qslc = qt_slice(qc)
                nc.tensor.matmul(
                    sc0[:],
                    kslc[0:64, :],
                    qslc[0:64, :],
                    start=True,
                    stop=True,
                    tile_position=(0, 0),
                )
                nc.tensor.matmul(
                    sc1[:],
                    kslc[64:128, :],
                    qslc[64:128, :],
                    start=True,
                    stop=True,
                    tile_position=(64, 0),
                )
                pt0 = ptp.tile([128, 512], U16, tag="pt0", name="pt0")
                pt1 = ptp.tile([128, 512], U16, tag="pt1", name="pt1")
                nc.scalar.activation(
                    pt0[:].bitcast(F16), sc0[:], EXP, bias=ebias[:], scale=SCALE
                )
                # fast exp: fp16 bits = round(SCH_A * s + SCH_B)
                nc.vector.tensor_scalar(pt1[:], sc1[:], SCH_A, SCH_B, MUL, ADD)
                return pt0, pt1

            def pv_step(kt, pts, pv0, pv1, n_last):
                pt0, pt1 = pts
                nc.tensor.matmul(
                    pv0[:],
                    v[kt][:, 0:65],
                    pt0[:].bitcast(F16),
                    start=(kt == 0),
                    stop=(kt == n_last),
                )
                nc.tensor.matmul(
                    pv1[:],
                    v[kt][:, 65:130],
                    pt1[:].bitcast(F16),
                    start=(kt == 0),
                    stop=(kt == n_last),
                )

            def drain(qc, pv0, pv1, smp):
                sums = smp.tile([1, 1024], F16, tag="sums", name="sums")
                # h0 drains on ScalarE, h1 on DVE (keeps both exp engines'
                # side-work symmetric)
                nc.scalar.copy(outT[qc][0:64, :], pv0[0:64, :])
                nc.scalar.copy(sums[0:1, 0:512], pv0[64:65, :])
                nc.vector.tensor_copy(outT[qc][64:128, :], pv1[0:64, :])
                nc.vector.tensor_copy(sums[0:1, 512:1024], pv1[64:65, :])
                return sums

            def proj_qtile(qc, qs, sums, smp, osp, psT):
                i = qc * 4 + qs
                ta = psT.tile([128, 512], F32, tag="ta", name="ta")
                tb = psT.tile([128, 512], F32, tag="tb", name="tb")
                # denominators -> partition layout via K=1 fp16 matmuls
                nc.tensor.matmul(
                    ta[:, 0:1],
                    sums[0:1, qs * 128 : (qs + 1) * 128],
                    ident16[0:1, 0:1],
                    start=True,
                    stop=True,
                )
                nc.tensor.matmul(
                    ta[:, 1:2],
                    sums[0:1, 512 + qs * 128 : 512 + (qs + 1) * 128],
                    ident16[0:1, 0:1],
                    start=True,
                    stop=True,
                )
                rc = smp.tile([128, 2], F32, tag="recip", name="rc")
                nc.vector.reciprocal(rc[:], ta[:, 0:2])
                oslc = outT[qc][:, qs * 128 : (qs + 1) * 128]
                nc.tensor.matmul(
                    ta[:],
                    oslc[0:64, :],
                    wp_sb[0:64, :],
                    start=True,
                    stop=True,
                    tile_position=(0, 0),
                )
                nc.tensor.matmul(
                    tb[:],
                    oslc[64:128, :],
                    wp_sb[64:128, :],
                    start=True,
                    stop=True,
                    tile_position=(64, 0),
                )
                t0 = osp.tile([128, 512], F32, tag="t0", name="t0")
                nc.scalar.activation(
                    t0[:], ta[:], mybir.ActivationFunctionType.Copy,
                    bias=0.0, scale=rc[:, 0:1],
                )
                ob = osp.tile([128, 512], F32, tag="ob", name="ob")
                nc.vector.scalar_tensor_tensor(
                    ob[:], tb[:], rc[:, 1:2], t0[:], op0=MUL, op1=ADD
                )
                nc.sync.dma_start(out=out[i * 128 : (i + 1) * 128, :], in_=ob[:])

            with tc.tile_pool(name="ptp", bufs=6) as ptp, tc.tile_pool(
                name="smp", bufs=2
            ) as smp, tc.tile_pool(name="osp", bufs=2) as osp, tc.tile_pool(
                name="psS", bufs=2, space="PSUM"
            ) as psS, tc.tile_pool(name="psV", bufs=1, space="PSUM") as psV:
                pv0_0 = psV.tile([65, 512], F32, tag="pv0", name="pv0")
                pv1_0 = psV.tile([65, 512], F32, tag="pv1", name="pv1")

                # ---- prologue: produce kT/qT/v per 1024-token group, with
                # qc=0's attention interleaved so the exp engines start early
                with tc.tile_pool(name="ldp", bufs=2) as ldp, tc.tile_pool(
                    name="psA", bufs=1, space="PSUM"
                ) as psA:
                    prev_pts = None
                    for g in range(n_g):
                        if g == 0:
                            xg = xg0
                        else:
                            xg = ldp.tile([128, 4096], F16, tag="xload", name="xg")
                            nc.sync.dma_start(
                                out=xg[:].rearrange("p (c w) -> p c w", c=4),
                                in_=xt[:, g * 1024 : (g + 1) * 1024].rearrange(
                                    "(c p) w -> p c w", c=4
                                ),
                            )
                        vts = ldp.tile([128, 1024], F16, tag="vts", name="vts")
                        for h in range(2):
                            hs = slice(h * 512, h * 512 + 512)
                            for w_sb, dst, copy_eng in (
                                (wk_sb, kT[g], nc.scalar.copy),
                                (wq_sb, qT[g], nc.scalar.copy),
                                (wv_sb, vts, nc.vector.tensor_copy),
                            ):
                                ps = psA.tile(
                                    [128, 512], F32, tag="work", name="ps_kqv"
                                )
                                for kc in range(4):
                                    nc.tensor.matmul(
                                        ps[:],
                                        w_sb[:, kc * 128 : (kc + 1) * 128],
                                        xg[:, kc * 1024 + h * 512 : kc * 1024 + h * 512 + 512],
                                        start=(kc == 0),
                                        stop=(kc == 3),
                                    )
                                copy_eng(dst[:, hs], ps[:])
                        for t in range(8 * g, 8 * g + 8):
                            j = t % 8
                            vps = psA.tile([128, 128], F16, tag="work", name="v_tr")
                            nc.tensor.transpose(
                                vps[:], vts[:, j * 128 : (j + 1) * 128], ident16[:]
                            )
                            # one strided copy scatters both head halves past
                            # the ones columns (cols 64/129 stay 1.0)
                            nc.vector.tensor_copy(
                                v[t][:, 0:130].rearrange("p (g w) -> p g w", g=2)[
                                    :, :, 0:64
                                ],
                                vps[:].rearrange("p (g w) -> p g w", g=2),
                            )
                        # qc=0 attention over this group's key tiles,
                        # software-pipelined: PV(kt-1) issues after
                        # scores(kt) so the PE queue never heads-of-line
                        # blocks on an exp result
                        for kt in range(8 * g, 8 * g + 8):
                            pts = scores_step(0, kt, psS, ptp)
                            if prev_pts is not None:
                                pv_step(kt - 1, prev_pts, pv0_0, pv1_0, n_xt - 1)
                            prev_pts = pts
                    pv_step(n_xt - 1, prev_pts, pv0_0, pv1_0, n_xt - 1)

                # ---- steady state: remaining chunks; each chunk's
                # projection is interleaved into the NEXT chunk's kt loop so
                # its PE/DVE work hides under the exp stream.
                with tc.tile_pool(name="psT", bufs=1, space="PSUM") as psT:
                    prev = drain(0, pv0_0, pv1_0, smp)
                    prev_qc = 0
                    for qc in range(1, n_qc):
                        pv0 = psV.tile([65, 512], F32, tag="pv0", name="pv0")
                        pv1 = psV.tile([65, 512], F32, tag="pv1", name="pv1")
                        prev_pts = None
                        for kt in range(n_xt):
                            pts = scores_step(qc, kt, psS, ptp)
                            if prev_pts is not None:
                                pv_step(kt - 1, prev_pts, pv0, pv1, n_xt - 1)
                            prev_pts = pts
                            if kt % 8 == 7 and kt // 8 < 3:
                                proj_qtile(prev_qc, kt // 8, prev, smp, osp, psT)
                        pv_step(n_xt - 1, prev_pts, pv0, pv1, n_xt - 1)
                        proj_qtile(prev_qc, 3, prev, smp, osp, psT)
                        prev = drain(qc, pv0, pv1, smp)
                        prev_qc = qc
                    # tail: hoist all denominator transposes + one recip so
                    # the 16 proj matmuls stream without per-qtile stalls
                    dn = psT.tile([128, 512], F32, tag="tb", name="dn")
                    for qs in range(4):
                        for h in range(2):
                            nc.tensor.matmul(
                                dn[:, 2 * qs + h : 2 * qs + h + 1],
                                prev[0:1, 512 * h + qs * 128 : 512 * h + (qs + 1) * 128],
                                ident16[0:1, 0:1],
                                start=True,
                                stop=True,
                            )
                    rca = smp.tile([128, 8], F32, tag="rcall", name="rca")
                    nc.vector.reciprocal(rca[:], dn[:, 0:8])
                    for qs in range(4):
                        i = prev_qc * 4 + qs
                        ta = psT.tile([128, 512], F32, tag="ta", name="ta")
                        tb = psT.tile([128, 512], F32, tag="tb", name="tb")
                        oslc = outT[prev_qc][:, qs * 128 : (qs + 1) * 128]
                        nc.tensor.matmul(
                            ta[:],
                            oslc[0:64, :],
                            wp_sb[0:64, :],
                            start=True,
                            stop=True,
                            tile_position=(0, 0),
                        )
                        nc.tensor.matmul(
                            tb[:],
                            oslc[64:128, :],
                            wp_sb[64:128, :],
                            start=True,
                            stop=True,
                            tile_position=(64, 0),
                        )
                        t0 = osp.tile([128, 512], F32, tag="t0", name="t0")
                        nc.scalar.activation(
                            t0[:], ta[:], mybir.ActivationFunctionType.Copy,
                            bias=0.0, scale=rca[:, 2 * qs : 2 * qs + 1],
                        )
                        ob = osp.tile([128, 512], F32, tag="ob", name="ob")
                        nc.vector.scalar_tensor_tensor(
                            ob[:], tb[:], rca[:, 2 * qs + 1 : 2 * qs + 2],
                            t0[:], op0=MUL, op1=ADD
                        )
                        nc.sync.dma_start(
                            out=out[i * 128 : (i + 1) * 128, :], in_=ob[:]
                        )
    nc.compile()
    return nc


_CACHE = {}


def _get_nc(tokens=N):
    if tokens not in _CACHE:
        _CACHE[tokens] = build(tokens)
    return _CACHE[tokens]


def _prep_w(w_slice):
    """[512, 128] -> [128, 512] fp16, layout w_[p, kc*128 + j] = w[kc*128 + p, j]."""
    w = np.asarray(w_slice, dtype=np.float16)
    return np.ascontiguousarray(
        w.reshape(4, 128, 128).transpose(1, 0, 2).reshape(128, 512)
    )


def _shard_inputs(x, w_qkv, w_proj):
    in_maps = []
    xt = [
        np.ascontiguousarray(x[b].T.astype(np.float16)) for b in range(x.shape[0])
    ]
    for c in range(8):
        b, hp = divmod(c, 4)
        o = 128 * hp
        in_maps.append(
            {
                "xt": xt[b],
                "wq": _prep_w(w_qkv[:, o : o + 128]),
                "wk": _prep_w(w_qkv[:, 512 + o : 512 + o + 128]),
                "wv": _prep_w(w_qkv[:, 1024 + o : 1024 + o + 128]),
                "wp": np.ascontiguousarray(
                    w_proj[o : o + 128, :], dtype=np.float16
                ),
            }
        )
    return in_maps


def run(x, w_qkv, w_proj, b_proj, trace=False, **kwargs):
    from concourse.bass_utils import run_bass_kernel_spmd

    nc = _get_nc()
    in_maps = _shard_inputs(
        np.asarray(x), np.asarray(w_qkv), np.asarray(w_proj)
    )
    br = run_bass_kernel_spmd(nc, in_maps, list(range(8)), trace=trace, **kwargs)
    parts = [np.asarray(br.results[c]["out"]) for c in range(8)]
    bp = np.asarray(b_proj)
    o0 = parts[0] + parts[1] + parts[2] + parts[3] + bp
    o1 = parts[4] + parts[5] + parts[6] + parts[7] + bp
    return np.stack([o0, o1]).astype(np.float32), br


def kernel(x, w_qkv, w_proj, b_proj):
    result, _ = run(x, w_qkv, w_proj, b_proj, trace=False)
    return result
